# revision 23
# baseline (speedup 1.0000x reference)
"""AttentionPooling Trainium2 kernel.

Problem: segment-softmax attention pooling over N=500000 nodes, H=256 features,
G=2048 graphs (sorted segment ids):
    h      = relu(x @ gate_w1 + gate_b1)            [N, 128]
    s      = (h @ gate_w2 + gate_b2)[:, 0]          [N]
    alpha  = segment_softmax(s, batch)              [N]
    feat   = relu(x @ feat_w + feat_b)              [N, 256]
    emb    = segment_sum(alpha[:, None] * feat)     [G, 256]
returns (emb, alpha).

Strategy (graph-level data parallel over 8 cores):
  * batch is sorted, so shard graphs contiguously: core c owns graphs
    [256c, 256(c+1)) and a contiguous node range. No collectives; host
    concatenates per-core outputs.
  * softmax max-subtraction is skipped (scores are O(1); exp never overflows;
    alpha is mathematically unchanged).
  * Single pass over x per core, nodes on partitions. Per 128-node tile:
      - PE (the bottleneck; this box runs the PE at 1.2 GHz):
        [h|feat] psum = xT_tile.T @ [gate_w1|feat_w] (bf16, 2 K-chunks),
        and a one-hot pool matmul accumulating U[g, :] += e_n * featr_n.
      - DVE: hfb = hf_psum + [b1|fb] broadcast (one tensor_tensor add),
        scores via scalar_tensor_tensor(max(hfb_h,0)*w2, accum_out).
      - ACT: e = exp(s + b2) batched over EB tiles;
        featr_e = Relu(e * hfb_feat) == e * relu(feat + fb)  (e > 0),
        including a constant-1.0 385th column of hfb so featr_e's last
        column is e itself -> pool's 257th column accumulates the softmax
        denominators for free.
      - The one-hot lhsT is e-free, so it is PRECOMPUTED ON HOST and DMA'd
        (graph window of 128 columns per tile; rare tiles spanning both
        128-graph chunks get an extra window appended at the end).
  * Host finishes with the O(G*H + N) division/gather: emb = U/den and
    alpha = e / den[batch] (pure unshard-time normalization).
"""

import math
import os

import ml_dtypes
import numpy as np

import concourse.bass as bass
import concourse.mybir as mybir
from concourse import bacc
import concourse.tile as tile
from concourse.bass_utils import run_bass_kernel_spmd

P = 128
H = 256
HF = 384          # h (128) + feat (256) fused output columns
G = 2048
NCORES = 8
GL = G // NCORES  # graphs per core
NB = 2048         # nodes per DMA macro block
TPM = NB // P     # node tiles per macro block
EB = 4            # exp batch (tiles per ACT exp instruction)
RING = 8          # hfb ring depth

BF16 = mybir.dt.bfloat16
F32 = mybir.dt.float32
BF16_NP = ml_dtypes.bfloat16

LAST_RESULT = None  # BassKernelResults of the most recent run (for test.py)


def _build(n_tiles: int, tile_chunks, extra_ids, b2: float) -> bass.Bass:
    """tile_chunks[t] = tuple of 128-graph chunk ids the tile touches
    (primary first). extra_ids[(t, ch)] = index into the extra one-hot
    window region for non-primary chunks."""
    nc = bacc.Bacc()
    n_pad = n_tiles * P
    n_macros = n_tiles // TPM
    n_extra = len(extra_ids)

    xt_d = nc.dram_tensor("xt", [H, n_pad], BF16, kind="ExternalInput")
    oh_d = nc.dram_tensor("oh", [P, n_pad], BF16, kind="ExternalInput")
    if n_extra:
        ohx_d = nc.dram_tensor("ohx", [P, n_extra * P], BF16, kind="ExternalInput")
    # packed constants:
    # cbf cols: [0:384]=wcat0, [384:768]=wcat1, [768:896]=ones/128 (dense),
    #           [896:1281]=[b1|fb|1.0] replicated on all rows
    cbf_d = nc.dram_tensor("cbf", [P, 1281], BF16, kind="ExternalInput")
    # cf32 cols: [0:128]=w2 broadcast, [128:256]=b1 broadcast, [256]=b2
    cf32_d = nc.dram_tensor("cf32", [P, 2 * P + 1], F32, kind="ExternalInput")

    e_out_d = nc.dram_tensor("e_out", [P, n_tiles], F32, kind="ExternalOutput")
    u_out_d = nc.dram_tensor("u_out", [2 * P, H + 1], F32, kind="ExternalOutput")

    first_use: dict[int, int] = {}
    last_use: dict[int, int] = {}
    for t, chs in enumerate(tile_chunks):
        for ch in chs:
            first_use.setdefault(ch, t)
            last_use[ch] = t

    relu = mybir.ActivationFunctionType.Relu
    expf = mybir.ActivationFunctionType.Exp
    op_max = mybir.AluOpType.max
    op_mult = mybir.AluOpType.mult
    op_add = mybir.AluOpType.add

    with tile.TileContext(nc) as tc:
        with (
            tc.tile_pool(name="const", bufs=1) as constp,
            tc.tile_pool(name="xt", bufs=3) as xtp,
            tc.tile_pool(name="work", bufs=4) as workp,
            tc.tile_pool(name="hf", bufs=4, space="PSUM") as hfp,
            tc.tile_pool(name="upsum", bufs=1, space="PSUM") as upp,
        ):
            cbf = constp.tile([P, 1281], BF16, tag="cbf")
            nc.sync.dma_start(cbf[:], cbf_d[:, :])
            cf32 = constp.tile([P, 2 * P + 1], F32, tag="cf32")
            nc.sync.dma_start(cf32[:], cf32_d[:, :])
            if n_extra:
                ohx = constp.tile([P, n_extra * P], BF16, tag="ohx")
                nc.sync.dma_start(ohx[:], ohx_d[:, :])
            wcat0 = cbf[:, 0:HF]
            wcat1 = cbf[:, HF : 2 * HF]
            onesb = cbf[:, 2 * HF : 2 * HF + P]
            bcat = cbf[:, 2 * HF + P : 2 * HF + P + HF + 1]
            w2b = cf32[:, 0:P]
            b1b = cf32[:, P : 2 * P]
            b2t = cf32[:, 2 * P : 2 * P + 1]
            e_all = constp.tile([P, n_tiles], F32, tag="e_all")
            s_all = constp.tile([P, n_tiles], F32, tag="s_all")
            sc = constp.tile([P, P], BF16, tag="sc")

            # Pre-join const DMA lanes into each engine's clock (keeps
            # per-instruction wait lists short).
            joinv = constp.tile([P, 1], F32, tag="joinv")
            nc.vector.tensor_copy(joinv[:], cf32[:, 0:1])
            joinv2 = constp.tile([P, 1], F32, tag="joinv2")
            nc.vector.tensor_copy(joinv2[:], cbf[:, 0:1])
            joina = constp.tile([P, 1], F32, tag="joina")
            nc.scalar.copy(joina[:], cf32[:, 0:1])

            upsum = {
                ch: upp.tile([P, H + 1], F32, tag=f"U{ch}", name=f"U{ch}")
                for ch in sorted(first_use)
            }

            featr_by_t: dict[int, object] = {}
            hf_by_t: dict[int, object] = {}
            pending_pool: list[tuple[int, object]] = []

            def emit_pool(t, oh_tile):
                featr = featr_by_t.pop(t)
                for ch in tile_chunks[t]:
                    if ch == tile_chunks[t][0]:
                        lhsT = oh_tile[:, (t % TPM) * P : (t % TPM + 1) * P]
                    else:
                        xi = extra_ids[(t, ch)]
                        lhsT = ohx[:, xi * P : (xi + 1) * P]
                    nc.tensor.matmul(
                        upsum[ch][:],
                        lhsT=lhsT,
                        rhs=featr[:],
                        start=(first_use[ch] == t),
                        stop=(last_use[ch] == t),
                        skip_group_check=True,
                    )

            oh_tiles = {}
            for m in range(n_macros):
                xt0 = xtp.tile([P, NB], BF16, tag="xt0")
                nc.sync.dma_start(xt0[:], xt_d[0:P, m * NB : (m + 1) * NB])
                xt1 = xtp.tile([P, NB], BF16, tag="xt1")
                nc.sync.dma_start(xt1[:], xt_d[P:H, m * NB : (m + 1) * NB])
                oh_t = xtp.tile([P, NB], BF16, tag="oh")
                nc.sync.dma_start(oh_t[:], oh_d[:, m * NB : (m + 1) * NB])
                oh_tiles[m] = oh_t
                for tt in range(TPM):
                    t = m * TPM + tt
                    sl = slice(tt * P, (tt + 1) * P)
                    hf = hfp.tile([P, HF + 1], F32, tag="hf", bufs=6)
                    nc.tensor.matmul(hf[:, 0:HF], lhsT=xt0[:, sl], rhs=wcat0[:], start=True, stop=False)
                    nc.tensor.matmul(hf[:, 0:HF], lhsT=xt1[:, sl], rhs=wcat1[:], start=False, stop=False)
                    nc.tensor.matmul(hf[:, P : HF + 1], lhsT=onesb[:], rhs=bcat[:, P : HF + 1], start=False, stop=True)

                    # drain one delayed pool-matmul group to keep PE dense
                    if pending_pool:
                        emit_pool(*pending_pool.pop(0))

                    hb = workp.tile([P, P], F32, tag="hb", bufs=4)
                    nc.vector.tensor_tensor(out=hb[:], in0=hf[:, 0:P], in1=b1b[:], op=op_add)
                    nc.vector.scalar_tensor_tensor(
                        out=sc[:], in0=hb[:], scalar=0.0, in1=w2b[:],
                        op0=op_max, op1=op_mult, accum_out=s_all[:, t : t + 1],
                    )

                    hf_by_t[t] = hf
                    if t % EB == EB - 1:
                        t0 = t - EB + 1
                        nc.scalar.activation(
                            e_all[:, t0 : t + 1], s_all[:, t0 : t + 1], expf, bias=b2t[:]
                        )
                        for tau in range(t0, t + 1):
                            if tile_chunks[tau]:
                                featr = workp.tile([P, H + 1], BF16, tag="featr", bufs=14)
                                nc.scalar.activation(
                                    featr[:],
                                    hf_by_t.pop(tau)[:, P : HF + 1],
                                    relu,
                                    scale=e_all[:, tau : tau + 1],
                                )
                                featr_by_t[tau] = featr
                                pending_pool.append((tau, oh_tiles[tau // TPM]))
                            else:
                                hf_by_t.pop(tau, None)

            for args in pending_pool:
                emit_pool(*args)
            oh_tiles.clear()

            for ch in (0, 1):
                u_sb = constp.tile([P, H + 1], F32, tag=f"usb{ch}", name=f"usb{ch}")
                if ch in upsum:
                    nc.vector.tensor_copy(u_sb[:], upsum[ch][:])
                else:
                    nc.vector.memset(u_sb[:], 0.0)
                nc.sync.dma_start(u_out_d[ch * P : (ch + 1) * P, :], u_sb[:])
            nc.sync.dma_start(e_out_d[:, :], e_all[:])

    nc.compile()
    return nc


def kernel(x, batch, gate_w1, gate_b1, gate_w2, gate_b2, feat_w, feat_b):
    global LAST_RESULT
    x = np.asarray(x, dtype=np.float32)
    batch = np.asarray(batch, dtype=np.int64)
    gate_w1 = np.asarray(gate_w1, dtype=np.float32)
    gate_b1 = np.asarray(gate_b1, dtype=np.float32)
    gate_w2 = np.asarray(gate_w2, dtype=np.float32)
    gate_b2 = np.asarray(gate_b2, dtype=np.float32)
    feat_w = np.asarray(feat_w, dtype=np.float32)
    feat_b = np.asarray(feat_b, dtype=np.float32)
    n = x.shape[0]

    bounds = np.searchsorted(batch, np.arange(0, G + 1, GL)).astype(np.int64)
    counts = np.diff(bounds)
    n_tiles = max(1, math.ceil(int(counts.max()) / P))
    n_tiles = math.ceil(n_tiles / TPM) * TPM
    n_pad = n_tiles * P

    x_bf = x.astype(BF16_NP)
    in_maps = []
    core_meta = []
    # per-tile chunk sets, unioned across cores (SPMD: one program)
    chunk_sets = [set() for _ in range(n_tiles)]
    core_bids = []
    for c in range(NCORES):
        s, e = int(bounds[c]), int(bounds[c + 1])
        cnt = e - s
        xt = np.zeros((H, n_pad), dtype=BF16_NP)
        xt[:, :cnt] = x_bf[s:e].T
        bid = np.full(n_pad, 2 * G, dtype=np.int32)
        bid[:cnt] = (batch[s:e] - c * GL).astype(np.int32)
        core_bids.append(bid)
        for t in range(n_tiles):
            ids = bid[t * P : (t + 1) * P]
            real = ids < 2 * P
            if real.any():
                lo = int(ids[real].min()) // P
                hi = int(ids[real].max()) // P
                chunk_sets[t].update(range(lo, hi + 1))
        core_meta.append((s, e, cnt))
        in_maps.append({"xt": xt})

    # order chunks per tile: primary = most common chunk across cores' nodes
    tile_chunks = []
    extra_ids = {}
    for t in range(n_tiles):
        chs = sorted(chunk_sets[t])
        if len(chs) > 1:
            # primary first (arbitrary but fixed); extras get appended windows
            for ch in chs[1:]:
                extra_ids[(t, ch)] = len(extra_ids)
        tile_chunks.append(tuple(chs))
    n_extra = len(extra_ids)

    # host-precomputed one-hot windows (e-free)
    for c in range(NCORES):
        bid = core_bids[c]
        oh = np.zeros((P, n_pad), dtype=BF16_NP)
        ohx = np.zeros((P, max(1, n_extra) * P), dtype=BF16_NP)
        cols = np.arange(P)
        for t in range(n_tiles):
            chs = tile_chunks[t]
            if not chs:
                continue
            ids = bid[t * P : (t + 1) * P]
            prim = chs[0]
            oh[:, t * P : (t + 1) * P] = (
                ids[:, None] == (prim * P + cols)[None, :]
            ).astype(BF16_NP)
            for ch in chs[1:]:
                xi = extra_ids[(t, ch)]
                ohx[:, xi * P : (xi + 1) * P] = (
                    ids[:, None] == (ch * P + cols)[None, :]
                ).astype(BF16_NP)
        in_maps[c]["oh"] = oh
        if n_extra:
            in_maps[c]["ohx"] = ohx

    wcat = np.concatenate([gate_w1, feat_w], axis=1).astype(BF16_NP)
    cbf = np.zeros((P, 1281), dtype=BF16_NP)
    cbf[:, 0:HF] = wcat[0:P]
    cbf[:, HF : 2 * HF] = wcat[P:H]
    cbf[:, 2 * HF : 2 * HF + P] = 1.0 / P
    cbf[:, 2 * HF + P : 2 * HF + P + HF] = np.concatenate([gate_b1, feat_b])[None, :].astype(BF16_NP)
    cbf[:, 2 * HF + P + HF] = 1.0
    cf32 = np.empty((P, 2 * P + 1), dtype=np.float32)
    cf32[:, 0:P] = gate_w2[:, 0][None, :]
    cf32[:, P : 2 * P] = gate_b1[None, :]
    cf32[:, 2 * P] = float(gate_b2[0])
    for m in in_maps:
        m.update(cbf=cbf, cf32=cf32)

    nc = _build(n_tiles, tile_chunks, extra_ids, float(gate_b2[0]))

    trace = bool(int(os.environ.get("KERNEL_TRACE", "0")))
    LAST_RESULT = run_bass_kernel_spmd(
        nc, in_maps, core_ids=list(range(NCORES)), trace=trace
    )
    results = LAST_RESULT.results

    emb = np.empty((G, H), dtype=np.float32)
    den = np.empty(G, dtype=np.float32)
    alpha = np.empty(n, dtype=np.float32)
    for c in range(NCORES):
        u = results[c]["u_out"]
        den_c = u[:, H]
        emb[c * GL : (c + 1) * GL] = u[:, :H] / np.maximum(den_c, 1e-30)[:, None]
        den[c * GL : (c + 1) * GL] = den_c
        s, e, cnt = core_meta[c]
        e_vals = results[c]["e_out"].T.reshape(-1)[:cnt]
        alpha[s:e] = e_vals / np.maximum(den[batch[s:e]], 1e-30)
    return emb, alpha


# revision 24
# speedup vs baseline: 1.3513x; 1.3513x over previous
"""AttentionPooling Trainium2 kernel.

Problem: segment-softmax attention pooling over N=500000 nodes, H=256 features,
G=2048 graphs (sorted segment ids):
    h      = relu(x @ gate_w1 + gate_b1)            [N, 128]
    s      = (h @ gate_w2 + gate_b2)[:, 0]          [N]
    alpha  = segment_softmax(s, batch)              [N]
    feat   = relu(x @ feat_w + feat_b)              [N, 256]
    emb    = segment_sum(alpha[:, None] * feat)     [G, 256]
returns (emb, alpha).

Strategy (graph-level data parallel over 8 cores):
  * batch is sorted, so shard graphs contiguously: core c owns graphs
    [256c, 256(c+1)) and a contiguous node range. No collectives; host
    concatenates per-core outputs.
  * softmax max-subtraction is skipped (scores are O(1); exp never overflows;
    alpha is mathematically unchanged).
  * Single pass over x per core, nodes on partitions. Per 128-node tile:
      - PE (the bottleneck; this box runs the PE at 1.2 GHz):
        [h|feat] psum = xT_tile.T @ [gate_w1|feat_w] (bf16, 2 K-chunks),
        and a one-hot pool matmul accumulating U[g, :] += e_n * featr_n.
      - DVE: hfb = hf_psum + [b1|fb] broadcast (one tensor_tensor add),
        scores via scalar_tensor_tensor(max(hfb_h,0)*w2, accum_out).
      - ACT: e = exp(s + b2) batched over EB tiles;
        featr_e = Relu(e * hfb_feat) == e * relu(feat + fb)  (e > 0),
        including a constant-1.0 385th column of hfb so featr_e's last
        column is e itself -> pool's 257th column accumulates the softmax
        denominators for free.
      - The one-hot lhsT is e-free, so it is PRECOMPUTED ON HOST and DMA'd
        (graph window of 128 columns per tile; rare tiles spanning both
        128-graph chunks get an extra window appended at the end).
  * Host finishes with the O(G*H + N) division/gather: emb = U/den and
    alpha = e / den[batch] (pure unshard-time normalization).
"""

import math
import os

import ml_dtypes
import numpy as np

import concourse.bass as bass
import concourse.mybir as mybir
from concourse import bacc
import concourse.tile as tile
from concourse.bass_utils import run_bass_kernel_spmd

P = 128
H = 256
HF = 384          # h (128) + feat (256) fused output columns
G = 2048
NCORES = 8
GL = G // NCORES  # graphs per core
NB = 2048         # nodes per DMA macro block
TPM = NB // P     # node tiles per macro block
EB = 4            # exp batch (tiles per ACT exp instruction)
RING = 8          # hfb ring depth

BF16 = mybir.dt.bfloat16
F32 = mybir.dt.float32
BF16_NP = ml_dtypes.bfloat16

LAST_RESULT = None  # BassKernelResults of the most recent run (for test.py)


def _build(n_tiles: int, tile_chunks, extra_ids, b2: float) -> bass.Bass:
    """tile_chunks[t] = tuple of 128-graph chunk ids the tile touches
    (primary first). extra_ids[(t, ch)] = index into the extra one-hot
    window region for non-primary chunks."""
    nc = bacc.Bacc()
    n_pad = n_tiles * P
    n_macros = n_tiles // TPM
    n_extra = len(extra_ids)

    xt_d = nc.dram_tensor("xt", [H, n_pad], BF16, kind="ExternalInput")
    oh_d = nc.dram_tensor("oh", [P, n_pad], BF16, kind="ExternalInput")
    if n_extra:
        ohx_d = nc.dram_tensor("ohx", [P, n_extra * P], BF16, kind="ExternalInput")
    # packed constants:
    # cbf cols: [0:384]=wcat0, [384:768]=wcat1
    cbf_d = nc.dram_tensor("cbf", [P, 2 * HF], BF16, kind="ExternalInput")
    # cf32 cols: [0:128]=w2 broadcast, [128:512]=bcat broadcast, [512]=b2
    cf32_d = nc.dram_tensor("cf32", [P, P + HF + 1], F32, kind="ExternalInput")

    e_out_d = nc.dram_tensor("e_out", [P, n_tiles], F32, kind="ExternalOutput")
    u_out_d = nc.dram_tensor("u_out", [2 * P, H + 1], F32, kind="ExternalOutput")

    first_use: dict[int, int] = {}
    last_use: dict[int, int] = {}
    for t, chs in enumerate(tile_chunks):
        for ch in chs:
            first_use.setdefault(ch, t)
            last_use[ch] = t

    relu = mybir.ActivationFunctionType.Relu
    expf = mybir.ActivationFunctionType.Exp
    op_max = mybir.AluOpType.max
    op_mult = mybir.AluOpType.mult
    op_add = mybir.AluOpType.add

    with tile.TileContext(nc) as tc:
        with (
            tc.tile_pool(name="const", bufs=1) as constp,
            tc.tile_pool(name="xt", bufs=3) as xtp,
            tc.tile_pool(name="work", bufs=4) as workp,
            tc.tile_pool(name="hf", bufs=4, space="PSUM") as hfp,
            tc.tile_pool(name="upsum", bufs=1, space="PSUM") as upp,
        ):
            cbf = constp.tile([P, 2 * HF], BF16, tag="cbf")
            nc.sync.dma_start(cbf[:], cbf_d[:, :])
            cf32 = constp.tile([P, P + HF + 1], F32, tag="cf32")
            nc.sync.dma_start(cf32[:], cf32_d[:, :])
            if n_extra:
                ohx = constp.tile([P, n_extra * P], BF16, tag="ohx")
                nc.sync.dma_start(ohx[:], ohx_d[:, :])
            wcat0 = cbf[:, 0:HF]
            wcat1 = cbf[:, HF : 2 * HF]
            w2b = cf32[:, 0:P]
            bcatb = cf32[:, P : P + HF]
            b2t = cf32[:, P + HF : P + HF + 1]
            e_all = constp.tile([P, n_tiles], F32, tag="e_all")
            s_all = constp.tile([P, n_tiles], F32, tag="s_all")
            sc = constp.tile([P, P], BF16, tag="sc")

            # hfb ring: biased [h|feat|1] tiles; column 384 stays 1.0 forever
            hfb_ring = [
                constp.tile([P, HF + 1], F32, tag=f"hfb{i}", name=f"hfb{i}")
                for i in range(RING)
            ]
            for hb in hfb_ring:
                nc.gpsimd.memset(hb[:, HF : HF + 1], 1.0)

            # Pre-join const DMA lanes into each engine's clock (keeps
            # per-instruction wait lists short).
            joinv = constp.tile([P, 1], F32, tag="joinv")
            nc.vector.tensor_copy(joinv[:], cf32[:, 0:1])
            joinv2 = constp.tile([P, 1], F32, tag="joinv2")
            nc.vector.tensor_copy(joinv2[:], cbf[:, 0:1])
            joina = constp.tile([P, 1], F32, tag="joina")
            nc.scalar.copy(joina[:], cf32[:, 0:1])

            upsum = {
                ch: upp.tile([P, H + 1], F32, tag=f"U{ch}", name=f"U{ch}")
                for ch in sorted(first_use)
            }

            featr_by_t: dict[int, object] = {}
            pending_pool: list[tuple[int, object]] = []

            def emit_pool(t, oh_tile):
                featr = featr_by_t.pop(t)
                for ch in tile_chunks[t]:
                    if ch == tile_chunks[t][0]:
                        lhsT = oh_tile[:, (t % TPM) * P : (t % TPM + 1) * P]
                    else:
                        xi = extra_ids[(t, ch)]
                        lhsT = ohx[:, xi * P : (xi + 1) * P]
                    nc.tensor.matmul(
                        upsum[ch][:],
                        lhsT=lhsT,
                        rhs=featr[:],
                        start=(first_use[ch] == t),
                        stop=(last_use[ch] == t),
                        skip_group_check=True,
                    )

            oh_tiles = {}
            for m in range(n_macros):
                xt0 = xtp.tile([P, NB], BF16, tag="xt0")
                nc.sync.dma_start(xt0[:], xt_d[0:P, m * NB : (m + 1) * NB])
                xt1 = xtp.tile([P, NB], BF16, tag="xt1")
                nc.sync.dma_start(xt1[:], xt_d[P:H, m * NB : (m + 1) * NB])
                oh_t = xtp.tile([P, NB], BF16, tag="oh")
                nc.sync.dma_start(oh_t[:], oh_d[:, m * NB : (m + 1) * NB])
                oh_tiles[m] = oh_t
                for tt in range(TPM):
                    t = m * TPM + tt
                    sl = slice(tt * P, (tt + 1) * P)
                    hf = hfp.tile([P, HF], F32, tag="hf")
                    nc.tensor.matmul(hf[:], lhsT=xt0[:, sl], rhs=wcat0[:], start=True, stop=False)
                    nc.tensor.matmul(hf[:], lhsT=xt1[:, sl], rhs=wcat1[:], start=False, stop=True)

                    # drain one delayed pool-matmul group to keep PE dense
                    if pending_pool:
                        emit_pool(*pending_pool.pop(0))

                    hfb = hfb_ring[t % RING]
                    nc.vector.tensor_tensor(
                        out=hfb[:, 0:HF], in0=hf[:], in1=bcatb[:], op=op_add
                    )
                    nc.vector.scalar_tensor_tensor(
                        out=sc[:], in0=hfb[:, 0:P], scalar=0.0, in1=w2b[:],
                        op0=op_max, op1=op_mult, accum_out=s_all[:, t : t + 1],
                    )

                    if t % EB == EB - 1:
                        t0 = t - EB + 1
                        nc.scalar.activation(
                            e_all[:, t0 : t + 1], s_all[:, t0 : t + 1], expf, bias=b2t[:]
                        )
                        for tau in range(t0, t + 1):
                            if tile_chunks[tau]:
                                featr = workp.tile([P, H + 1], BF16, tag="featr", bufs=14)
                                nc.scalar.activation(
                                    featr[:],
                                    hfb_ring[tau % RING][:, P : HF + 1],
                                    relu,
                                    scale=e_all[:, tau : tau + 1],
                                )
                                featr_by_t[tau] = featr
                                pending_pool.append((tau, oh_tiles[tau // TPM]))

            for args in pending_pool:
                emit_pool(*args)
            oh_tiles.clear()

            for ch in (0, 1):
                u_sb = constp.tile([P, H + 1], F32, tag=f"usb{ch}", name=f"usb{ch}")
                if ch in upsum:
                    nc.vector.tensor_copy(u_sb[:], upsum[ch][:])
                else:
                    nc.vector.memset(u_sb[:], 0.0)
                nc.sync.dma_start(u_out_d[ch * P : (ch + 1) * P, :], u_sb[:])
            nc.sync.dma_start(e_out_d[:, :], e_all[:])

    nc.compile()
    return nc


def kernel(x, batch, gate_w1, gate_b1, gate_w2, gate_b2, feat_w, feat_b):
    global LAST_RESULT
    x = np.asarray(x, dtype=np.float32)
    batch = np.asarray(batch, dtype=np.int64)
    gate_w1 = np.asarray(gate_w1, dtype=np.float32)
    gate_b1 = np.asarray(gate_b1, dtype=np.float32)
    gate_w2 = np.asarray(gate_w2, dtype=np.float32)
    gate_b2 = np.asarray(gate_b2, dtype=np.float32)
    feat_w = np.asarray(feat_w, dtype=np.float32)
    feat_b = np.asarray(feat_b, dtype=np.float32)
    n = x.shape[0]

    bounds = np.searchsorted(batch, np.arange(0, G + 1, GL)).astype(np.int64)
    counts = np.diff(bounds)
    n_tiles = max(1, math.ceil(int(counts.max()) / P))
    n_tiles = math.ceil(n_tiles / TPM) * TPM
    n_pad = n_tiles * P

    x_bf = x.astype(BF16_NP)
    in_maps = []
    core_meta = []
    # per-tile chunk sets, unioned across cores (SPMD: one program)
    chunk_sets = [set() for _ in range(n_tiles)]
    core_bids = []
    for c in range(NCORES):
        s, e = int(bounds[c]), int(bounds[c + 1])
        cnt = e - s
        xt = np.zeros((H, n_pad), dtype=BF16_NP)
        xt[:, :cnt] = x_bf[s:e].T
        bid = np.full(n_pad, 2 * G, dtype=np.int32)
        bid[:cnt] = (batch[s:e] - c * GL).astype(np.int32)
        core_bids.append(bid)
        for t in range(n_tiles):
            ids = bid[t * P : (t + 1) * P]
            real = ids < 2 * P
            if real.any():
                lo = int(ids[real].min()) // P
                hi = int(ids[real].max()) // P
                chunk_sets[t].update(range(lo, hi + 1))
        core_meta.append((s, e, cnt))
        in_maps.append({"xt": xt})

    # order chunks per tile: primary = most common chunk across cores' nodes
    tile_chunks = []
    extra_ids = {}
    for t in range(n_tiles):
        chs = sorted(chunk_sets[t])
        if len(chs) > 1:
            # primary first (arbitrary but fixed); extras get appended windows
            for ch in chs[1:]:
                extra_ids[(t, ch)] = len(extra_ids)
        tile_chunks.append(tuple(chs))
    n_extra = len(extra_ids)

    # host-precomputed one-hot windows (e-free)
    for c in range(NCORES):
        bid = core_bids[c]
        oh = np.zeros((P, n_pad), dtype=BF16_NP)
        ohx = np.zeros((P, max(1, n_extra) * P), dtype=BF16_NP)
        cols = np.arange(P)
        for t in range(n_tiles):
            chs = tile_chunks[t]
            if not chs:
                continue
            ids = bid[t * P : (t + 1) * P]
            prim = chs[0]
            oh[:, t * P : (t + 1) * P] = (
                ids[:, None] == (prim * P + cols)[None, :]
            ).astype(BF16_NP)
            for ch in chs[1:]:
                xi = extra_ids[(t, ch)]
                ohx[:, xi * P : (xi + 1) * P] = (
                    ids[:, None] == (ch * P + cols)[None, :]
                ).astype(BF16_NP)
        in_maps[c]["oh"] = oh
        if n_extra:
            in_maps[c]["ohx"] = ohx

    wcat = np.concatenate([gate_w1, feat_w], axis=1).astype(BF16_NP)
    cbf = np.zeros((P, 2 * HF), dtype=BF16_NP)
    cbf[:, 0:HF] = wcat[0:P]
    cbf[:, HF : 2 * HF] = wcat[P:H]
    cf32 = np.empty((P, P + HF + 1), dtype=np.float32)
    cf32[:, 0:P] = gate_w2[:, 0][None, :]
    cf32[:, P : P + HF] = np.concatenate([gate_b1, feat_b])[None, :]
    cf32[:, P + HF] = float(gate_b2[0])
    for m in in_maps:
        m.update(cbf=cbf, cf32=cf32)

    nc = _build(n_tiles, tile_chunks, extra_ids, float(gate_b2[0]))

    trace = bool(int(os.environ.get("KERNEL_TRACE", "0")))
    LAST_RESULT = run_bass_kernel_spmd(
        nc, in_maps, core_ids=list(range(NCORES)), trace=trace
    )
    results = LAST_RESULT.results

    emb = np.empty((G, H), dtype=np.float32)
    den = np.empty(G, dtype=np.float32)
    alpha = np.empty(n, dtype=np.float32)
    for c in range(NCORES):
        u = results[c]["u_out"]
        den_c = u[:, H]
        emb[c * GL : (c + 1) * GL] = u[:, :H] / np.maximum(den_c, 1e-30)[:, None]
        den[c * GL : (c + 1) * GL] = den_c
        s, e, cnt = core_meta[c]
        e_vals = results[c]["e_out"].T.reshape(-1)[:cnt]
        alpha[s:e] = e_vals / np.maximum(den[batch[s:e]], 1e-30)
    return emb, alpha


# revision 25
# speedup vs baseline: 1.4898x; 1.1025x over previous
"""AttentionPooling Trainium2 kernel.

Problem: segment-softmax attention pooling over N=500000 nodes, H=256 features,
G=2048 graphs (sorted segment ids):
    h      = relu(x @ gate_w1 + gate_b1)            [N, 128]
    s      = (h @ gate_w2 + gate_b2)[:, 0]          [N]
    alpha  = segment_softmax(s, batch)              [N]
    feat   = relu(x @ feat_w + feat_b)              [N, 256]
    emb    = segment_sum(alpha[:, None] * feat)     [G, 256]
returns (emb, alpha).

Strategy (graph-level data parallel over 8 cores):
  * batch is sorted, so shard graphs contiguously: core c owns graphs
    [256c, 256(c+1)) and a contiguous node range. No collectives; host
    concatenates per-core outputs.
  * softmax max-subtraction is skipped (scores are O(1); exp never overflows;
    alpha is mathematically unchanged).
  * Single pass over x per core, nodes on partitions. Per 128-node tile:
      - PE (the bottleneck; this box runs the PE at 1.2 GHz):
        [h|feat] psum = xT_tile.T @ [gate_w1|feat_w] (bf16, 2 K-chunks),
        and a one-hot pool matmul accumulating U[g, :] += e_n * featr_n.
      - DVE: hfb = hf_psum + [b1|fb] broadcast (one tensor_tensor add),
        scores via scalar_tensor_tensor(max(hfb_h,0)*w2, accum_out).
      - ACT: e = exp(s + b2) batched over EB tiles;
        featr_e = Relu(e * hfb_feat) == e * relu(feat + fb)  (e > 0),
        including a constant-1.0 385th column of hfb so featr_e's last
        column is e itself -> pool's 257th column accumulates the softmax
        denominators for free.
      - The one-hot lhsT is e-free, so it is PRECOMPUTED ON HOST and DMA'd
        (graph window of 128 columns per tile; rare tiles spanning both
        128-graph chunks get an extra window appended at the end).
  * Host finishes with the O(G*H + N) division/gather: emb = U/den and
    alpha = e / den[batch] (pure unshard-time normalization).
"""

import math
import os

import ml_dtypes
import numpy as np

import concourse.bass as bass
import concourse.mybir as mybir
from concourse import bacc
import concourse.tile as tile
from concourse.bass_utils import run_bass_kernel_spmd

P = 128
H = 256
HF = 384          # h (128) + feat (256) fused output columns
G = 2048
NCORES = 8
GL = G // NCORES  # graphs per core
NB = 2048         # nodes per DMA macro block
TPM = NB // P     # node tiles per macro block
EB = 4            # exp batch (tiles per ACT exp instruction)
RING = 8          # hfb ring depth

BF16 = mybir.dt.bfloat16
F32 = mybir.dt.float32
BF16_NP = ml_dtypes.bfloat16

LAST_RESULT = None  # BassKernelResults of the most recent run (for test.py)


def _build(n_tiles: int, tile_chunks, extra_ids, b2: float) -> bass.Bass:
    """tile_chunks[t] = tuple of 128-graph chunk ids the tile touches
    (primary first). extra_ids[(t, ch)] = index into the extra one-hot
    window region for non-primary chunks."""
    nc = bacc.Bacc()
    n_pad = n_tiles * P
    n_macros = n_tiles // TPM
    n_extra = len(extra_ids)

    xt_d = nc.dram_tensor("xt", [H, n_pad], BF16, kind="ExternalInput")
    oh_d = nc.dram_tensor("oh", [P, n_pad], BF16, kind="ExternalInput")
    if n_extra:
        ohx_d = nc.dram_tensor("ohx", [P, n_extra * P], BF16, kind="ExternalInput")
    # packed constants:
    # cbf cols: [0:384]=wcat0, [384:768]=wcat1, [768:896]=ones/128 dense,
    #           [896:1024]=b1 replicated rows
    cbf_d = nc.dram_tensor("cbf", [P, 2 * HF + 2 * P], BF16, kind="ExternalInput")
    # cf32 cols: [0:128]=w2 broadcast, [128:384]=feat_b broadcast, [384]=b2
    cf32_d = nc.dram_tensor("cf32", [P, P + H + 1], F32, kind="ExternalInput")

    e_out_d = nc.dram_tensor("e_out", [P, n_tiles], F32, kind="ExternalOutput")
    u_out_d = nc.dram_tensor("u_out", [2 * P, H + 1], F32, kind="ExternalOutput")

    first_use: dict[int, int] = {}
    last_use: dict[int, int] = {}
    for t, chs in enumerate(tile_chunks):
        for ch in chs:
            first_use.setdefault(ch, t)
            last_use[ch] = t

    relu = mybir.ActivationFunctionType.Relu
    expf = mybir.ActivationFunctionType.Exp
    op_max = mybir.AluOpType.max
    op_mult = mybir.AluOpType.mult
    op_add = mybir.AluOpType.add

    with tile.TileContext(nc) as tc:
        with (
            tc.tile_pool(name="const", bufs=1) as constp,
            tc.tile_pool(name="xt", bufs=3) as xtp,
            tc.tile_pool(name="work", bufs=4) as workp,
            tc.tile_pool(name="hf", bufs=4, space="PSUM") as hfp,
            tc.tile_pool(name="upsum", bufs=1, space="PSUM") as upp,
        ):
            cbf = constp.tile([P, 2 * HF + 2 * P], BF16, tag="cbf")
            nc.sync.dma_start(cbf[:], cbf_d[:, :])
            cf32 = constp.tile([P, P + H + 1], F32, tag="cf32")
            nc.sync.dma_start(cf32[:], cf32_d[:, :])
            if n_extra:
                ohx = constp.tile([P, n_extra * P], BF16, tag="ohx")
                nc.sync.dma_start(ohx[:], ohx_d[:, :])
            wcat0 = cbf[:, 0:HF]
            wcat1 = cbf[:, HF : 2 * HF]
            onesd = cbf[:, 2 * HF : 2 * HF + P]
            b1rep = cbf[:, 2 * HF + P : 2 * HF + 2 * P]
            w2b = cf32[:, 0:P]
            fbb = cf32[:, P : P + H]
            b2t = cf32[:, P + H : P + H + 1]
            e_all = constp.tile([P, n_tiles], F32, tag="e_all")
            s_all = constp.tile([P, n_tiles], F32, tag="s_all")
            sc = constp.tile([P, P], BF16, tag="sc")

            # hfb ring: biased [feat|1] tiles; column 256 stays 1.0 forever
            hfb_ring = [
                constp.tile([P, H + 1], F32, tag=f"hfb{i}", name=f"hfb{i}")
                for i in range(RING)
            ]
            for hb in hfb_ring:
                nc.gpsimd.memset(hb[:, H : H + 1], 1.0)

            # Pre-join const DMA lanes into each engine's clock (keeps
            # per-instruction wait lists short).
            joinv = constp.tile([P, 1], F32, tag="joinv")
            nc.vector.tensor_copy(joinv[:], cf32[:, 0:1])
            joinv2 = constp.tile([P, 1], F32, tag="joinv2")
            nc.vector.tensor_copy(joinv2[:], cbf[:, 0:1])
            joina = constp.tile([P, 1], F32, tag="joina")
            nc.scalar.copy(joina[:], cf32[:, 0:1])

            upsum = {
                ch: upp.tile([P, H + 1], F32, tag=f"U{ch}", name=f"U{ch}")
                for ch in sorted(first_use)
            }

            featr_by_t: dict[int, object] = {}
            pending_pool: list[tuple[int, object]] = []

            def emit_pool(t, oh_tile):
                featr = featr_by_t.pop(t)
                for ch in tile_chunks[t]:
                    if ch == tile_chunks[t][0]:
                        lhsT = oh_tile[:, (t % TPM) * P : (t % TPM + 1) * P]
                    else:
                        xi = extra_ids[(t, ch)]
                        lhsT = ohx[:, xi * P : (xi + 1) * P]
                    nc.tensor.matmul(
                        upsum[ch][:],
                        lhsT=lhsT,
                        rhs=featr[:],
                        start=(first_use[ch] == t),
                        stop=(last_use[ch] == t),
                        skip_group_check=True,
                    )

            oh_tiles = {}
            for m in range(n_macros):
                xt0 = xtp.tile([P, NB], BF16, tag="xt0")
                nc.sync.dma_start(xt0[:], xt_d[0:P, m * NB : (m + 1) * NB])
                xt1 = xtp.tile([P, NB], BF16, tag="xt1")
                nc.sync.dma_start(xt1[:], xt_d[P:H, m * NB : (m + 1) * NB])
                oh_t = xtp.tile([P, NB], BF16, tag="oh")
                nc.sync.dma_start(oh_t[:], oh_d[:, m * NB : (m + 1) * NB])
                oh_tiles[m] = oh_t
                for tt in range(TPM):
                    t = m * TPM + tt
                    sl = slice(tt * P, (tt + 1) * P)
                    hf = hfp.tile([P, HF], F32, tag="hf")
                    nc.tensor.matmul(hf[:], lhsT=xt0[:, sl], rhs=wcat0[:], start=True, stop=False)
                    nc.tensor.matmul(hf[:], lhsT=xt1[:, sl], rhs=wcat1[:], start=False, stop=False)
                    nc.tensor.matmul(hf[:, 0:P], lhsT=onesd[:], rhs=b1rep[:], start=False, stop=True, skip_group_check=True)

                    # drain one delayed pool-matmul group to keep PE dense
                    if pending_pool:
                        emit_pool(*pending_pool.pop(0))

                    hfb = hfb_ring[t % RING]
                    nc.vector.tensor_tensor(
                        out=hfb[:, 0:H], in0=hf[:, P:HF], in1=fbb[:], op=op_add
                    )
                    nc.vector.scalar_tensor_tensor(
                        out=sc[:], in0=hf[:, 0:P], scalar=0.0, in1=w2b[:],
                        op0=op_max, op1=op_mult, accum_out=s_all[:, t : t + 1],
                    )

                    if t % EB == EB - 1:
                        t0 = t - EB + 1
                        nc.scalar.activation(
                            e_all[:, t0 : t + 1], s_all[:, t0 : t + 1], expf, bias=b2t[:]
                        )
                        for tau in range(t0, t + 1):
                            if tile_chunks[tau]:
                                featr = workp.tile([P, H + 1], BF16, tag="featr", bufs=14)
                                nc.scalar.activation(
                                    featr[:],
                                    hfb_ring[tau % RING][:],
                                    relu,
                                    scale=e_all[:, tau : tau + 1],
                                )
                                featr_by_t[tau] = featr
                                pending_pool.append((tau, oh_tiles[tau // TPM]))

            for args in pending_pool:
                emit_pool(*args)
            oh_tiles.clear()

            for ch in (0, 1):
                u_sb = constp.tile([P, H + 1], F32, tag=f"usb{ch}", name=f"usb{ch}")
                if ch in upsum:
                    nc.vector.tensor_copy(u_sb[:], upsum[ch][:])
                else:
                    nc.vector.memset(u_sb[:], 0.0)
                nc.sync.dma_start(u_out_d[ch * P : (ch + 1) * P, :], u_sb[:])
            nc.sync.dma_start(e_out_d[:, :], e_all[:])

    nc.compile()
    return nc


def kernel(x, batch, gate_w1, gate_b1, gate_w2, gate_b2, feat_w, feat_b):
    global LAST_RESULT
    x = np.asarray(x, dtype=np.float32)
    batch = np.asarray(batch, dtype=np.int64)
    gate_w1 = np.asarray(gate_w1, dtype=np.float32)
    gate_b1 = np.asarray(gate_b1, dtype=np.float32)
    gate_w2 = np.asarray(gate_w2, dtype=np.float32)
    gate_b2 = np.asarray(gate_b2, dtype=np.float32)
    feat_w = np.asarray(feat_w, dtype=np.float32)
    feat_b = np.asarray(feat_b, dtype=np.float32)
    n = x.shape[0]

    bounds = np.searchsorted(batch, np.arange(0, G + 1, GL)).astype(np.int64)
    counts = np.diff(bounds)
    n_tiles = max(1, math.ceil(int(counts.max()) / P))
    n_tiles = math.ceil(n_tiles / TPM) * TPM
    n_pad = n_tiles * P

    x_bf = x.astype(BF16_NP)
    in_maps = []
    core_meta = []
    # per-tile chunk sets, unioned across cores (SPMD: one program)
    chunk_sets = [set() for _ in range(n_tiles)]
    core_bids = []
    for c in range(NCORES):
        s, e = int(bounds[c]), int(bounds[c + 1])
        cnt = e - s
        xt = np.zeros((H, n_pad), dtype=BF16_NP)
        xt[:, :cnt] = x_bf[s:e].T
        bid = np.full(n_pad, 2 * G, dtype=np.int32)
        bid[:cnt] = (batch[s:e] - c * GL).astype(np.int32)
        core_bids.append(bid)
        for t in range(n_tiles):
            ids = bid[t * P : (t + 1) * P]
            real = ids < 2 * P
            if real.any():
                lo = int(ids[real].min()) // P
                hi = int(ids[real].max()) // P
                chunk_sets[t].update(range(lo, hi + 1))
        core_meta.append((s, e, cnt))
        in_maps.append({"xt": xt})

    # order chunks per tile: primary = most common chunk across cores' nodes
    tile_chunks = []
    extra_ids = {}
    for t in range(n_tiles):
        chs = sorted(chunk_sets[t])
        if len(chs) > 1:
            # primary first (arbitrary but fixed); extras get appended windows
            for ch in chs[1:]:
                extra_ids[(t, ch)] = len(extra_ids)
        tile_chunks.append(tuple(chs))
    n_extra = len(extra_ids)

    # host-precomputed one-hot windows (e-free)
    for c in range(NCORES):
        bid = core_bids[c]
        oh = np.zeros((P, n_pad), dtype=BF16_NP)
        ohx = np.zeros((P, max(1, n_extra) * P), dtype=BF16_NP)
        cols = np.arange(P)
        for t in range(n_tiles):
            chs = tile_chunks[t]
            if not chs:
                continue
            ids = bid[t * P : (t + 1) * P]
            prim = chs[0]
            oh[:, t * P : (t + 1) * P] = (
                ids[:, None] == (prim * P + cols)[None, :]
            ).astype(BF16_NP)
            for ch in chs[1:]:
                xi = extra_ids[(t, ch)]
                ohx[:, xi * P : (xi + 1) * P] = (
                    ids[:, None] == (ch * P + cols)[None, :]
                ).astype(BF16_NP)
        in_maps[c]["oh"] = oh
        if n_extra:
            in_maps[c]["ohx"] = ohx

    wcat = np.concatenate([gate_w1, feat_w], axis=1).astype(BF16_NP)
    cbf = np.zeros((P, 2 * HF + 2 * P), dtype=BF16_NP)
    cbf[:, 0:HF] = wcat[0:P]
    cbf[:, HF : 2 * HF] = wcat[P:H]
    cbf[:, 2 * HF : 2 * HF + P] = 1.0 / P
    cbf[:, 2 * HF + P : 2 * HF + 2 * P] = gate_b1[None, :].astype(BF16_NP)
    cf32 = np.empty((P, P + H + 1), dtype=np.float32)
    cf32[:, 0:P] = gate_w2[:, 0][None, :]
    cf32[:, P : P + H] = feat_b[None, :]
    cf32[:, P + H] = float(gate_b2[0])
    for m in in_maps:
        m.update(cbf=cbf, cf32=cf32)

    nc = _build(n_tiles, tile_chunks, extra_ids, float(gate_b2[0]))

    trace = bool(int(os.environ.get("KERNEL_TRACE", "0")))
    LAST_RESULT = run_bass_kernel_spmd(
        nc, in_maps, core_ids=list(range(NCORES)), trace=trace
    )
    results = LAST_RESULT.results

    emb = np.empty((G, H), dtype=np.float32)
    den = np.empty(G, dtype=np.float32)
    alpha = np.empty(n, dtype=np.float32)
    for c in range(NCORES):
        u = results[c]["u_out"]
        den_c = u[:, H]
        emb[c * GL : (c + 1) * GL] = u[:, :H] / np.maximum(den_c, 1e-30)[:, None]
        den[c * GL : (c + 1) * GL] = den_c
        s, e, cnt = core_meta[c]
        e_vals = results[c]["e_out"].T.reshape(-1)[:cnt]
        alpha[s:e] = e_vals / np.maximum(den[batch[s:e]], 1e-30)
    return emb, alpha


# revision 26
# speedup vs baseline: 1.6212x; 1.0881x over previous
"""AttentionPooling Trainium2 kernel.

Problem: segment-softmax attention pooling over N=500000 nodes, H=256 features,
G=2048 graphs (sorted segment ids):
    h      = relu(x @ gate_w1 + gate_b1)            [N, 128]
    s      = (h @ gate_w2 + gate_b2)[:, 0]          [N]
    alpha  = segment_softmax(s, batch)              [N]
    feat   = relu(x @ feat_w + feat_b)              [N, 256]
    emb    = segment_sum(alpha[:, None] * feat)     [G, 256]
returns (emb, alpha).

Strategy (graph-level data parallel over 8 cores):
  * batch is sorted, so shard graphs contiguously: core c owns graphs
    [256c, 256(c+1)) and a contiguous node range. No collectives; host
    concatenates per-core outputs.
  * softmax max-subtraction is skipped (scores are O(1); exp never overflows;
    alpha is mathematically unchanged).
  * Single pass over x per core, nodes on partitions. Per 128-node tile:
      - PE (the bottleneck; this box runs the PE at 1.2 GHz):
        [h|feat] psum = xT_tile.T @ [gate_w1|feat_w] (bf16, 2 K-chunks),
        and a one-hot pool matmul accumulating U[g, :] += e_n * featr_n.
      - DVE: hfb = hf_psum + [b1|fb] broadcast (one tensor_tensor add),
        scores via scalar_tensor_tensor(max(hfb_h,0)*w2, accum_out).
      - ACT: e = exp(s + b2) batched over EB tiles;
        featr_e = Relu(e * hfb_feat) == e * relu(feat + fb)  (e > 0),
        including a constant-1.0 385th column of hfb so featr_e's last
        column is e itself -> pool's 257th column accumulates the softmax
        denominators for free.
      - The one-hot lhsT is e-free, so it is PRECOMPUTED ON HOST and DMA'd
        (graph window of 128 columns per tile; rare tiles spanning both
        128-graph chunks get an extra window appended at the end).
  * Host finishes with the O(G*H + N) division/gather: emb = U/den and
    alpha = e / den[batch] (pure unshard-time normalization).
"""

import math
import os

import ml_dtypes
import numpy as np

import concourse.bass as bass
import concourse.mybir as mybir
from concourse import bacc
import concourse.tile as tile
from concourse.bass_utils import run_bass_kernel_spmd

P = 128
H = 256
HF = 384          # h (128) + feat (256) fused output columns
G = 2048
NCORES = 8
GL = G // NCORES  # graphs per core
NB = 2048         # nodes per DMA macro block
TPM = NB // P     # node tiles per macro block
EB = 8            # exp batch (tiles per ACT exp instruction)
RING = 12         # hfb ring depth

BF16 = mybir.dt.bfloat16
F32 = mybir.dt.float32
BF16_NP = ml_dtypes.bfloat16

LAST_RESULT = None  # BassKernelResults of the most recent run (for test.py)


def _build(n_tiles: int, tile_chunks, extra_ids, b2: float) -> bass.Bass:
    """tile_chunks[t] = tuple of 128-graph chunk ids the tile touches
    (primary first). extra_ids[(t, ch)] = index into the extra one-hot
    window region for non-primary chunks."""
    nc = bacc.Bacc()
    n_pad = n_tiles * P
    n_macros = n_tiles // TPM
    n_extra = len(extra_ids)

    xt_d = nc.dram_tensor("xt", [H, n_pad], BF16, kind="ExternalInput")
    oh_d = nc.dram_tensor("oh", [P, n_pad], BF16, kind="ExternalInput")
    if n_extra:
        ohx_d = nc.dram_tensor("ohx", [P, n_extra * P], BF16, kind="ExternalInput")
    # packed constants:
    # cbf cols: [0:384]=wcat0, [384:768]=wcat1, [768:896]=ones/128 dense,
    #           [896:1024]=b1 replicated rows
    cbf_d = nc.dram_tensor("cbf", [P, 2 * HF + 2 * P], BF16, kind="ExternalInput")
    # cf32 cols: [0:128]=w2 broadcast, [128:384]=feat_b broadcast, [384]=b2
    cf32_d = nc.dram_tensor("cf32", [P, P + H + 1], F32, kind="ExternalInput")

    e_out_d = nc.dram_tensor("e_out", [P, n_tiles], F32, kind="ExternalOutput")
    u_out_d = nc.dram_tensor("u_out", [2 * P, H + 1], F32, kind="ExternalOutput")

    first_use: dict[int, int] = {}
    last_use: dict[int, int] = {}
    for t, chs in enumerate(tile_chunks):
        for ch in chs:
            first_use.setdefault(ch, t)
            last_use[ch] = t

    relu = mybir.ActivationFunctionType.Relu
    expf = mybir.ActivationFunctionType.Exp
    op_max = mybir.AluOpType.max
    op_mult = mybir.AluOpType.mult
    op_add = mybir.AluOpType.add

    with tile.TileContext(nc) as tc:
        with (
            tc.tile_pool(name="const", bufs=1) as constp,
            tc.tile_pool(name="xt", bufs=4) as xtp,
            tc.tile_pool(name="work", bufs=4) as workp,
            tc.tile_pool(name="hf", bufs=4, space="PSUM") as hfp,
            tc.tile_pool(name="upsum", bufs=1, space="PSUM") as upp,
        ):
            cbf = constp.tile([P, 2 * HF + 2 * P], BF16, tag="cbf")
            nc.sync.dma_start(cbf[:], cbf_d[:, :])
            cf32 = constp.tile([P, P + H + 1], F32, tag="cf32")
            nc.sync.dma_start(cf32[:], cf32_d[:, :])
            if n_extra:
                ohx = constp.tile([P, n_extra * P], BF16, tag="ohx")
                nc.sync.dma_start(ohx[:], ohx_d[:, :])
            wcat0 = cbf[:, 0:HF]
            wcat1 = cbf[:, HF : 2 * HF]
            onesd = cbf[:, 2 * HF : 2 * HF + P]
            b1rep = cbf[:, 2 * HF + P : 2 * HF + 2 * P]
            w2b = cf32[:, 0:P]
            fbb = cf32[:, P : P + H]
            b2t = cf32[:, P + H : P + H + 1]
            e_all = constp.tile([P, n_tiles], F32, tag="e_all")
            s_all = constp.tile([P, n_tiles], F32, tag="s_all")
            sc = constp.tile([P, P], BF16, tag="sc")

            # hfb ring: biased [feat|1] tiles; column 256 stays 1.0 forever
            hfb_ring = [
                constp.tile([P, H + 1], F32, tag=f"hfb{i}", name=f"hfb{i}")
                for i in range(RING)
            ]
            for hb in hfb_ring:
                nc.gpsimd.memset(hb[:, H : H + 1], 1.0)

            # Pre-join const DMA lanes into each engine's clock (keeps
            # per-instruction wait lists short).
            joinv = constp.tile([P, 1], F32, tag="joinv")
            nc.vector.tensor_copy(joinv[:], cf32[:, 0:1])
            joinv2 = constp.tile([P, 1], F32, tag="joinv2")
            nc.vector.tensor_copy(joinv2[:], cbf[:, 0:1])
            joina = constp.tile([P, 1], F32, tag="joina")
            nc.scalar.copy(joina[:], cf32[:, 0:1])

            upsum = {
                ch: upp.tile([P, H + 1], F32, tag=f"U{ch}", name=f"U{ch}")
                for ch in sorted(first_use)
            }

            featr_by_t: dict[int, object] = {}
            pending_pool: list[tuple[int, object]] = []

            def emit_pool(t, oh_tile):
                featr = featr_by_t.pop(t)
                for ch in tile_chunks[t]:
                    if ch == tile_chunks[t][0]:
                        lhsT = oh_tile[:, (t % TPM) * P : (t % TPM + 1) * P]
                    else:
                        xi = extra_ids[(t, ch)]
                        lhsT = ohx[:, xi * P : (xi + 1) * P]
                    nc.tensor.matmul(
                        upsum[ch][:],
                        lhsT=lhsT,
                        rhs=featr[:],
                        start=(first_use[ch] == t),
                        stop=(last_use[ch] == t),
                        skip_group_check=True,
                    )

            oh_tiles = {}
            for m in range(n_macros):
                xt0 = xtp.tile([P, NB], BF16, tag="xt0")
                nc.sync.dma_start(xt0[:], xt_d[0:P, m * NB : (m + 1) * NB])
                xt1 = xtp.tile([P, NB], BF16, tag="xt1")
                nc.sync.dma_start(xt1[:], xt_d[P:H, m * NB : (m + 1) * NB])
                oh_t = xtp.tile([P, NB], BF16, tag="oh")
                nc.sync.dma_start(oh_t[:], oh_d[:, m * NB : (m + 1) * NB])
                oh_tiles[m] = oh_t
                for tt in range(TPM):
                    t = m * TPM + tt
                    sl = slice(tt * P, (tt + 1) * P)
                    hf = hfp.tile([P, HF], F32, tag="hf", bufs=6)
                    nc.tensor.matmul(hf[:], lhsT=xt0[:, sl], rhs=wcat0[:], start=True, stop=False)
                    nc.tensor.matmul(hf[:], lhsT=xt1[:, sl], rhs=wcat1[:], start=False, stop=False)
                    nc.tensor.matmul(hf[:, 0:P], lhsT=onesd[:], rhs=b1rep[:], start=False, stop=True, skip_group_check=True)

                    # drain one delayed pool-matmul group to keep PE dense
                    if pending_pool:
                        emit_pool(*pending_pool.pop(0))

                    hfb = hfb_ring[t % RING]
                    nc.vector.tensor_tensor(
                        out=hfb[:, 0:H], in0=hf[:, P:HF], in1=fbb[:], op=op_add
                    )
                    nc.vector.scalar_tensor_tensor(
                        out=sc[:], in0=hf[:, 0:P], scalar=0.0, in1=w2b[:],
                        op0=op_max, op1=op_mult, accum_out=s_all[:, t : t + 1],
                    )

                    if t % EB == EB - 1:
                        t0 = t - EB + 1
                        nc.scalar.activation(
                            e_all[:, t0 : t + 1], s_all[:, t0 : t + 1], expf, bias=b2t[:]
                        )
                        for tau in range(t0, t + 1):
                            if tile_chunks[tau]:
                                featr = workp.tile([P, H + 1], BF16, tag="featr", bufs=20)
                                nc.scalar.activation(
                                    featr[:],
                                    hfb_ring[tau % RING][:],
                                    relu,
                                    scale=e_all[:, tau : tau + 1],
                                )
                                featr_by_t[tau] = featr
                                pending_pool.append((tau, oh_tiles[tau // TPM]))

            for args in pending_pool:
                emit_pool(*args)
            oh_tiles.clear()

            for ch in (0, 1):
                u_sb = constp.tile([P, H + 1], F32, tag=f"usb{ch}", name=f"usb{ch}")
                if ch in upsum:
                    nc.vector.tensor_copy(u_sb[:], upsum[ch][:])
                else:
                    nc.vector.memset(u_sb[:], 0.0)
                nc.sync.dma_start(u_out_d[ch * P : (ch + 1) * P, :], u_sb[:])
            nc.sync.dma_start(e_out_d[:, :], e_all[:])

    nc.compile()
    return nc


def kernel(x, batch, gate_w1, gate_b1, gate_w2, gate_b2, feat_w, feat_b):
    global LAST_RESULT
    x = np.asarray(x, dtype=np.float32)
    batch = np.asarray(batch, dtype=np.int64)
    gate_w1 = np.asarray(gate_w1, dtype=np.float32)
    gate_b1 = np.asarray(gate_b1, dtype=np.float32)
    gate_w2 = np.asarray(gate_w2, dtype=np.float32)
    gate_b2 = np.asarray(gate_b2, dtype=np.float32)
    feat_w = np.asarray(feat_w, dtype=np.float32)
    feat_b = np.asarray(feat_b, dtype=np.float32)
    n = x.shape[0]

    bounds = np.searchsorted(batch, np.arange(0, G + 1, GL)).astype(np.int64)
    counts = np.diff(bounds)
    n_tiles = max(1, math.ceil(int(counts.max()) / P))
    n_tiles = math.ceil(n_tiles / TPM) * TPM
    n_pad = n_tiles * P

    x_bf = x.astype(BF16_NP)
    in_maps = []
    core_meta = []
    # per-tile chunk sets, unioned across cores (SPMD: one program)
    chunk_sets = [set() for _ in range(n_tiles)]
    core_bids = []
    for c in range(NCORES):
        s, e = int(bounds[c]), int(bounds[c + 1])
        cnt = e - s
        xt = np.zeros((H, n_pad), dtype=BF16_NP)
        xt[:, :cnt] = x_bf[s:e].T
        bid = np.full(n_pad, 2 * G, dtype=np.int32)
        bid[:cnt] = (batch[s:e] - c * GL).astype(np.int32)
        core_bids.append(bid)
        for t in range(n_tiles):
            ids = bid[t * P : (t + 1) * P]
            real = ids < 2 * P
            if real.any():
                lo = int(ids[real].min()) // P
                hi = int(ids[real].max()) // P
                chunk_sets[t].update(range(lo, hi + 1))
        core_meta.append((s, e, cnt))
        in_maps.append({"xt": xt})

    # order chunks per tile: primary = most common chunk across cores' nodes
    tile_chunks = []
    extra_ids = {}
    for t in range(n_tiles):
        chs = sorted(chunk_sets[t])
        if len(chs) > 1:
            # primary first (arbitrary but fixed); extras get appended windows
            for ch in chs[1:]:
                extra_ids[(t, ch)] = len(extra_ids)
        tile_chunks.append(tuple(chs))
    n_extra = len(extra_ids)

    # host-precomputed one-hot windows (e-free)
    for c in range(NCORES):
        bid = core_bids[c]
        oh = np.zeros((P, n_pad), dtype=BF16_NP)
        ohx = np.zeros((P, max(1, n_extra) * P), dtype=BF16_NP)
        cols = np.arange(P)
        for t in range(n_tiles):
            chs = tile_chunks[t]
            if not chs:
                continue
            ids = bid[t * P : (t + 1) * P]
            prim = chs[0]
            oh[:, t * P : (t + 1) * P] = (
                ids[:, None] == (prim * P + cols)[None, :]
            ).astype(BF16_NP)
            for ch in chs[1:]:
                xi = extra_ids[(t, ch)]
                ohx[:, xi * P : (xi + 1) * P] = (
                    ids[:, None] == (ch * P + cols)[None, :]
                ).astype(BF16_NP)
        in_maps[c]["oh"] = oh
        if n_extra:
            in_maps[c]["ohx"] = ohx

    wcat = np.concatenate([gate_w1, feat_w], axis=1).astype(BF16_NP)
    cbf = np.zeros((P, 2 * HF + 2 * P), dtype=BF16_NP)
    cbf[:, 0:HF] = wcat[0:P]
    cbf[:, HF : 2 * HF] = wcat[P:H]
    cbf[:, 2 * HF : 2 * HF + P] = 1.0 / P
    cbf[:, 2 * HF + P : 2 * HF + 2 * P] = gate_b1[None, :].astype(BF16_NP)
    cf32 = np.empty((P, P + H + 1), dtype=np.float32)
    cf32[:, 0:P] = gate_w2[:, 0][None, :]
    cf32[:, P : P + H] = feat_b[None, :]
    cf32[:, P + H] = float(gate_b2[0])
    for m in in_maps:
        m.update(cbf=cbf, cf32=cf32)

    nc = _build(n_tiles, tile_chunks, extra_ids, float(gate_b2[0]))

    trace = bool(int(os.environ.get("KERNEL_TRACE", "0")))
    LAST_RESULT = run_bass_kernel_spmd(
        nc, in_maps, core_ids=list(range(NCORES)), trace=trace
    )
    results = LAST_RESULT.results

    emb = np.empty((G, H), dtype=np.float32)
    den = np.empty(G, dtype=np.float32)
    alpha = np.empty(n, dtype=np.float32)
    for c in range(NCORES):
        u = results[c]["u_out"]
        den_c = u[:, H]
        emb[c * GL : (c + 1) * GL] = u[:, :H] / np.maximum(den_c, 1e-30)[:, None]
        den[c * GL : (c + 1) * GL] = den_c
        s, e, cnt = core_meta[c]
        e_vals = results[c]["e_out"].T.reshape(-1)[:cnt]
        alpha[s:e] = e_vals / np.maximum(den[batch[s:e]], 1e-30)
    return emb, alpha
